# revision 12
# baseline (speedup 1.0000x reference)
"""Trainium2 Bass kernel for nn_ASDHead (dense_mlp).

Math (per batch item b, one NeuronCore each):
    f_proj = features[b] @ W_f                      # (T=1024, H=128)
    s_proj = slots[b] @ W_s + b_proj                # (N=64,  H=128)
    out[b, t, n] = sum_h relu(f_proj[t,h] + s_proj[n,h]) * w_head[h] + b_head

Sharding: data-parallel over B (8 batch items -> 8 NeuronCores), weights
replicated. Host pre-transposes so the contraction dim is on partitions.

Per-core schedule (DVE+ACT saturated on the x elementwise ops, which is the
floor for this decomposition; measured per-op rates on this
hardware in situ: DVE tensor_scalar with per-partition scalar ptr ~280 ns,
ACT activation relu+bias ~510 ns; split 48:16 with all PSUM evacuations
on ACT):
  - f_projT (h=128p, t=1024) and s_projT via PE matmuls; f_proj cast to bf16.
  - 64 slots: x_n = relu(f_projT + s_projT[:, n]) as one per-partition-bias
    op [128, 1024] bf16, split greedily between DVE and ACT.
  - Dense-packed PE reduction: each (slot n, t-half th) strip is reduced by
    a matmul whose stationary is a zero-padded [128, 32] pattern
    (col j = w_head), so the strip lands on PSUM partition row p = 2n+th.
    Col group g accumulates its 32 strips into rows 32g..32g+32 of its own
    PSUM bank (separate banks so the four concurrent matmul streams never
    share a bank write port), double-buffered across iterations (8 banks).
    Four per-group [32, 512] evacuation ops (+b_head) fill one staging tile,
    then a single 256 KB DMA ships the whole (N, T) output.
  - Features are shipped bf16 (half the upload bytes), t-half-major so the
    first f_proj matmuls overlap the second half's DMA.
"""

import numpy as np
from contextlib import ExitStack

B, T, N = 8, 1024, 64
D_MODEL, D_SLOT, H = 256, 256, 128
P = 128
TH = T // 512  # 2 t-halves per slot

_CACHE = {}


class _Split:
    """Greedy engine picker by virtual finish time (costs in ns)."""

    def __init__(self):
        self.t = {"dve": 0.0, "act": 0.0}

    def pick(self, dve_cost, act_cost):
        if self.t["dve"] + dve_cost <= self.t["act"] + act_cost:
            self.t["dve"] += dve_cost
            return "dve"
        self.t["act"] += act_cost
        return "act"


# measured per-op costs (ns) on this hardware, pipelined back-to-back
COST_X_DVE = 400.0  # tensor_scalar ptr bf16 [128,1024], pipelined in situ
COST_X_ACT = 1200.0  # activation relu+bias [128,1024], pipelined in situ (split 48:16)
# evacuations go to ACT: DVE is the binding engine for the x ops, and the
# measured split with all four evacs on ACT is fastest
COST_EVAC_DVE = 1e9
COST_EVAC_ACT = 1.0


def _build_bass(repeat=1, costs=None, xbufs=12, evac_costs=None):
    import concourse.mybir as mybir
    import concourse.tile as tile
    from concourse import bacc

    f32 = mybir.dt.float32
    f32r = mybir.dt.float32r
    bf16 = mybir.dt.bfloat16
    Alu = mybir.AluOpType
    Act = mybir.ActivationFunctionType

    nc = bacc.Bacc()

    featT = nc.dram_tensor("featT", (D_MODEL, T), bf16, kind="ExternalInput")
    slotT = nc.dram_tensor("slotT", (D_SLOT, N), f32, kind="ExternalInput")
    wf = nc.dram_tensor("wf", (D_MODEL, H), bf16, kind="ExternalInput")
    ws = nc.dram_tensor("ws", (D_SLOT, H), f32, kind="ExternalInput")
    bproj = nc.dram_tensor("bproj", (H, 1), f32, kind="ExternalInput")
    bhead = nc.dram_tensor("bhead", (P, 1), f32, kind="ExternalInput")
    # zero-padded stationary patterns: w32[h, j, m] = w_head[h] if m==j else 0
    w32 = nc.dram_tensor("w32", (H, 32, 32), bf16, kind="ExternalInput")
    out = nc.dram_tensor("out", (N, T), f32, kind="ExternalOutput")

    with tile.TileContext(nc) as tc, ExitStack() as ctx:
        pctx = ctx.enter_context(ExitStack())
        singles = ctx.enter_context(tc.tile_pool(name="singles", bufs=1))
        xpool = ctx.enter_context(tc.tile_pool(name="xpool", bufs=xbufs))
        stage_pool = ctx.enter_context(tc.tile_pool(name="stage", bufs=2))

        # ---- load inputs (d on partitions, 2 chunks of 128) ----
        # features arrive t-half-major so the th=0 f_proj matmuls can start
        # while the th=1 half is still in flight
        featT_sb = singles.tile([P, 2, T], bf16)
        featT_v = featT.rearrange("(c p) t -> p c t", p=P)
        for th in range(TH):
            nc.sync.dma_start(
                featT_sb[:, :, th * 512 : (th + 1) * 512],
                featT_v[:, :, th * 512 : (th + 1) * 512],
            )
        slotT_sb = singles.tile([P, 2, N], f32r)
        nc.sync.dma_start(slotT_sb, slotT.rearrange("(c p) n -> p c n", p=P).bitcast(f32r))
        wf_sb = singles.tile([P, 2, H], bf16)
        nc.sync.dma_start(wf_sb, wf.rearrange("(c p) h -> p c h", p=P))
        ws_sb = singles.tile([P, 2, H], f32r)
        nc.sync.dma_start(ws_sb, ws.rearrange("(c p) h -> p c h", p=P).bitcast(f32r))
        bproj_sb = singles.tile([P, 1], f32)
        nc.sync.dma_start(bproj_sb, bproj[:, :])
        bhead_sb = singles.tile([P, 1], f32)
        nc.sync.dma_start(bhead_sb, bhead[:, :])
        w32_sb = singles.tile([P, 32, 32], bf16)
        nc.sync.dma_start(w32_sb, w32[:, :, :])

        # ---- s_projT (h=128p, n=64) = W_s.T @ slotsT + b_proj ----
        mm_psum = pctx.enter_context(tc.tile_pool(name="mm_psum", bufs=1, space="PSUM"))
        sp_ps_full = mm_psum.tile([P, 512], f32, tag="mm", name="sp_ps")
        sp_ps = sp_ps_full[:, :N]
        for c in range(2):
            nc.tensor.matmul(
                sp_ps, ws_sb[:, c], slotT_sb[:, c], start=(c == 0), stop=(c == 1)
            )
        sp_sb = singles.tile([P, N], f32)
        nc.scalar.activation(sp_sb, sp_ps, Act.Identity, bias=bproj_sb, scale=1.0)

        # ---- f_projT (h=128p, t=1024) = W_f.T @ featT -> bf16 ----
        fp_bf = singles.tile([P, T], bf16)
        fp_ps = mm_psum.tile([P, 2, 512], f32, tag="mm", name="fp_ps")
        for th in range(TH):
            for c in range(2):
                nc.tensor.matmul(
                    fp_ps[:, th],
                    wf_sb[:, c],
                    featT_sb[:, c, th * 512 : (th + 1) * 512],
                    start=(c == 0),
                    stop=(c == 1),
                )
        # two half copies PSUM->SBUF with bf16 cast, one per engine, so the
        # first x-ops can start as soon as their half lands
        nc.vector.tensor_copy(fp_bf[:, :512], fp_ps[:, 0])
        nc.scalar.copy(fp_bf[:, 512:], fp_ps[:, 1])
        pctx.close()  # release prologue PSUM banks for the reduction pool
        red_psum = ctx.enter_context(tc.tile_pool(name="red_psum", bufs=2, space="PSUM"))

        split = _Split()
        c_dve, c_act = costs or (COST_X_DVE, COST_X_ACT)
        out_v = out.rearrange("n (th c) -> (n th) c", th=TH)

        # ---- main loop ----
        for it in range(repeat):
            # four PSUM banks (one per col group, so the four matmul streams
            # never share a bank write port); group g accumulates its 32
            # strips into rows 32g..32g+31 of bank g. Double-buffered across
            # iterations (2 x 4 = all 8 banks).
            psum_red = red_psum.tile([P, 4, 512], f32, tag="red", name="psum_red")
            staging = stage_pool.tile([P, 512], f32, tag="stg", name="staging")
            o_gs = [psum_red[32 * g : 32 * (g + 1), g] for g in range(4)]

            for r in range(16):
                xs = []
                for g in range(4):
                    n = 16 * g + r
                    x = xpool.tile([P, T], bf16, tag="x", name="x")
                    halves = (
                        [(th * 512, (th + 1) * 512) for th in range(TH)]
                        if (it == 0 and r == 0)
                        else [(0, T)]
                    )
                    for lo, hi in halves:
                        frac = (hi - lo) / T
                        if split.pick(c_dve * frac, c_act * frac) == "dve":
                            nc.vector.tensor_scalar(
                                out=x[:, lo:hi],
                                in0=fp_bf[:, lo:hi],
                                scalar1=sp_sb[:, n : n + 1],
                                scalar2=0.0,
                                op0=Alu.add,
                                op1=Alu.max,
                            )
                        else:
                            nc.scalar.activation(
                                x[:, lo:hi],
                                fp_bf[:, lo:hi],
                                Act.Relu,
                                bias=sp_sb[:, n : n + 1],
                                scale=1.0,
                            )
                    xs.append((g, x))

                for th in range(TH):
                    for g, x in xs:
                        j = 2 * r + th
                        nc.tensor.matmul(
                            o_gs[g],
                            w32_sb[:, j],
                            x[:, th * 512 : (th + 1) * 512],
                            start=(j == 0),
                            stop=(j == 31),
                            tile_position=(0, 32 * g),
                        )

            # four per-group evacuations (+b_head) into one staging tile,
            # then a single 256 KB output DMA
            ec_dve, ec_act = evac_costs or (COST_EVAC_DVE, COST_EVAC_ACT)
            for g in range(4):
                sg = staging[32 * g : 32 * (g + 1), :]
                bh = bhead_sb[32 * g : 32 * (g + 1)]
                if split.pick(ec_dve, ec_act) == "dve":
                    nc.vector.tensor_scalar(
                        out=sg, in0=o_gs[g], scalar1=bh, scalar2=None, op0=Alu.add
                    )
                else:
                    nc.scalar.activation(sg, o_gs[g], Act.Identity, bias=bh, scale=1.0)
            nc.sync.dma_start(out_v, staging)

    nc.finalize()
    return nc


def kernel(features, slots, W_proj, b_proj, w_head, b_head):
    import ml_dtypes
    from concourse.bass_utils import run_bass_kernel_spmd

    if "nc" not in _CACHE:
        _CACHE["nc"] = _build_bass()
    nc = _CACHE["nc"]

    features = np.asarray(features, dtype=np.float32)
    slots = np.asarray(slots, dtype=np.float32)
    W_proj = np.asarray(W_proj, dtype=np.float32)
    b_proj = np.asarray(b_proj, dtype=np.float32)
    w_head = np.asarray(w_head, dtype=np.float32)
    b_head = np.asarray(b_head, dtype=np.float32)

    bf16 = ml_dtypes.bfloat16
    wf = np.ascontiguousarray(W_proj[:D_MODEL]).astype(bf16)  # (256, 128)
    ws = np.ascontiguousarray(W_proj[D_MODEL:])  # (256, 128)
    bproj = np.ascontiguousarray(b_proj.reshape(H, 1))
    bhead = np.full((P, 1), b_head, dtype=np.float32)
    w32 = np.zeros((H, 32, 32), dtype=bf16)
    w32[:, np.arange(32), np.arange(32)] = w_head[:, None].astype(bf16)

    in_maps = []
    for b in range(B):
        in_maps.append(
            {
                "featT": np.ascontiguousarray(features[b].T).astype(bf16),
                "slotT": np.ascontiguousarray(slots[b].T),
                "wf": wf,
                "ws": ws,
                "bproj": bproj,
                "bhead": bhead,
                "w32": w32,
            }
        )
    _CACHE["in_maps"] = in_maps

    # transient device wedges (NRT_EXEC_UNIT_UNRECOVERABLE) clear on re-run
    last_err = None
    for _attempt in range(2):
        try:
            res = run_bass_kernel_spmd(nc, in_maps, core_ids=list(range(B)))
            # per-core out is (N, T); assemble (B, T, N)
            out = np.stack([r["out"].T for r in res.results], axis=0)
            return out.astype(np.float32)
        except Exception as e:  # noqa: BLE001
            last_err = e
    raise last_err
